# revision 65
# baseline (speedup 1.0000x reference)
"""Bass/Trainium2 kernel for nn_Attention_369367188096 (sparse_attention).

Reference computation (B=2, N=4096, IN_DIM=1024, DIM=1024, HEADS=8, d=128):
    qkv = x @ W_qkv ; split into q,k,v per head
    dots = (q @ k^T) * DIM**-0.5 ; masked on top-left [2048,2048] block
    attn = softmax(dots) ; out = attn @ v ; out @ W_out + b_out

Sharding across 8 NeuronCores: core i handles batch b=i//4 and heads
(2*(i%4), 2*(i%4)+1).  Each core computes a partial output
x[b]-rows x DIM using its two heads' slice of W_out (row-sharded);
the host sums 4 partials per batch and adds b_out.

PE-bound design (~390us vs 528us bf16 baseline); every non-S matmul
stream shrunk and all engines kept busy end-to-end:
- On real TRN2 a matmul costs out-free-size cycles regardless of dtype;
  fp8 DoubleRow's win is contracting TWO 128-deep k-planes per stream.
  PV and the softmax denominator contract j (4096) -> DR pairs halve
  them; Q/K projections contract IN_DIM (1024) -> DR over c-chunk pairs
  (x and W_q/W_k shipped as fp8).  V projection keeps bf16 W_v against
  the fp8 x (mixed-dtype matmul) so V only carries x's quantization.
  In fully-unmasked groups the denominator folds once more: DVE pre-sums
  each exp pair in fp8 so one ones-weights DR matmul covers 4 j-chunks.
- S = K^T Q contracts only d=128, so it stays bf16 (no DR win exists).
- exp on ScalarE writes fp8 directly; mask (resident in SBUF) is an fp8
  0/1 multiply on VectorE; 1/den via DVE reciprocal_approx_fast.
- One flat 256-pair attention stream with S-emission two pairs ahead
  (exp runs back-to-back through group boundaries); V-projection chunks,
  head-1 Q/K projection units and output-projection halves are spliced
  into the pair loop's PE slack (in-order engine queues make emission
  order = execution order), so the PE never idles long enough to drop
  out of its top p-state.
- Input DMAs are sliced per c-chunk/n-quarter (a single dma_start runs
  on one ~22GB/s engine) and the issue stream is split across the Sync
  and Scalar HWDGE queues; output partials ship as bf16.
"""

import os
import sys

for _p in ("/opt/trn_rl_repo", "/root/.axon_site/_ro/trn_rl_repo"):
    if os.path.isdir(_p) and _p not in sys.path:
        sys.path.insert(0, _p)

from collections import deque
from contextlib import ExitStack

import ml_dtypes
import numpy as np

import concourse.bass as bass
import concourse.bacc as bacc
import concourse.mybir as mybir
import concourse.tile as tile
from concourse.bass_utils import run_bass_kernel_spmd

BF16 = mybir.dt.bfloat16
FP8 = mybir.dt.float8e4
F32 = mybir.dt.float32
DR = mybir.MatmulPerfMode.DoubleRow
P = 128          # partitions
IN_DIM = 1024    # model in dim
OUT_DIM = 1024   # model out dim
DH = 128         # head dim
NH = 2           # heads per core
FD = 512         # matmul moving free dim
N_FULL = 4096    # sequence length
MM_FULL = 2048   # masked block size
SCALE = 1024 ** -0.5
N_CORES = 8


def build_nc(n=N_FULL, mm=MM_FULL):
    """Build the per-core Bass program (SPMD: same program, per-core data)."""
    CI = IN_DIM // P          # 8 input-dim chunks
    CP = CI // 2              # c-chunk pairs for DR projections (4)
    JC = n // P               # key chunks (32)
    IG = n // FD              # query groups of 512 (8)
    MJ = mm // P              # masked key chunks (16)
    MG = mm // FD             # masked query groups (4)
    assert MJ % 2 == 0 and JC % 2 == 0
    AF = mybir.ActivationFunctionType

    nc = bacc.Bacc("TRN2", target_bir_lowering=False, debug=False)
    wq_d = nc.dram_tensor("wq", [P, CI * NH * DH], FP8, kind="ExternalInput")
    wk_d = nc.dram_tensor("wk", [P, CI * NH * DH], FP8, kind="ExternalInput")
    wv_d = nc.dram_tensor("wv", [P, CI * NH * DH], BF16, kind="ExternalInput")
    wo_d = nc.dram_tensor("wo", [P, NH * OUT_DIM], BF16, kind="ExternalInput")
    x8_d = nc.dram_tensor("x8", [IN_DIM, n], FP8, kind="ExternalInput")
    mk_d = nc.dram_tensor("maskt", [mm, mm], FP8, kind="ExternalInput")
    out_d = nc.dram_tensor("part", [n, OUT_DIM], BF16, kind="ExternalOutput")

    NQ = n // 4               # x8 DMA quarter width
    x8_v = x8_d.rearrange("(c p) n -> c p n", p=P)
    mk_v = mk_d.rearrange("(j p) i -> p j i", p=P)
    out_v = out_d.rearrange("(t p) o -> t p o", p=P)

    with tile.TileContext(nc) as tc, ExitStack() as ctx:
        const = ctx.enter_context(tc.tile_pool(name="const", bufs=1))

        # Resident inputs. Transfers are sliced small so they parallelize
        # across the 16 DMA engines (a single dma_start runs on ONE engine
        # at ~22GB/s), and the issue stream is split across the two HWDGE
        # queues (Sync + Scalar) because each dma_start costs ~0.6us of
        # issue time on its queue.  Sync: W slices + x8 first half.
        # Scalar (idle until the first exp): x8 second half.
        # x8 lives as four separate n-quarter tiles: the Tile dependency
        # tracker is per-tile, so a consumer of quarter q must not be made
        # to wait on later quarters' transfers.
        # W and x8 live as c-PAIR tiles -- one tile per DoubleRow matmul
        # operand -- so each DR matmul only waits on its own two c-chunks
        # (the dependency tracker is per-tile).
        wq8 = [const.tile([P, 2, NH * DH], FP8, tag=f"wqp{i}", name=f"wqp{i}")
               for i in range(CP)]
        wk8 = [const.tile([P, 2, NH * DH], FP8, tag=f"wkp{i}", name=f"wkp{i}")
               for i in range(CP)]
        wq_v = wq_d.rearrange("p (a b) -> p a b", a=CI)
        wk_v = wk_d.rearrange("p (a b) -> p a b", a=CI)
        x8q = [[const.tile([P, 2, NQ], FP8, tag=f"x8p{q}{i}",
                           name=f"x8p{q}{i}") for i in range(CP)]
               for q in range(4)]

        def dma_x8(eng, q, c):
            eng.dma_start(x8q[q][c // 2][:, c % 2, :],
                          x8_v[c][:, q * NQ:(q + 1) * NQ])

        wv = const.tile([P, CI, NH * DH], BF16, tag="wv")
        wo = const.tile([P, NH, OUT_DIM], BF16, tag="wo")
        wv_v = wv_d.rearrange("p (a b) -> p a b", a=CI)
        # Sync queue: interleave wq8 pair-tiles with x8q0's first c-slices
        # so the very first projection matmuls unblock ~15us in; Scalar
        # queue (idle until the first exp) carries wk8 and the rest.
        nc.sync.dma_start(wq8[0][:], wq_v[:, 0:2, :])
        dma_x8(nc.sync, 0, 0)
        dma_x8(nc.sync, 0, 1)
        nc.sync.dma_start(wq8[1][:], wq_v[:, 2:4, :])
        dma_x8(nc.sync, 0, 2)
        dma_x8(nc.sync, 0, 3)
        nc.sync.dma_start(wq8[2][:], wq_v[:, 4:6, :])
        nc.sync.dma_start(wq8[3][:], wq_v[:, 6:8, :])
        dma_x8(nc.scalar, 0, 4)
        dma_x8(nc.scalar, 0, 5)
        nc.scalar.dma_start(wk8[0][:], wk_v[:, 0:2, :])
        dma_x8(nc.scalar, 0, 6)
        dma_x8(nc.scalar, 0, 7)
        for i in range(1, CP):
            nc.scalar.dma_start(wk8[i][:], wk_v[:, 2 * i:2 * i + 2, :])
        for c in range(CI):
            dma_x8(nc.sync if c < 4 else nc.scalar, 1, c)
        for c in range(0, CI, 2):
            nc.sync.dma_start(wv[:, c:c + 2, :], wv_v[:, c:c + 2, :])
        for c in range(CI):
            dma_x8(nc.sync if c < 4 else nc.scalar, 2, c)
        for c in range(CI):
            dma_x8(nc.sync if c < 4 else nc.scalar, 3, c)
        nc.sync.dma_start(wo[:], wo_d.rearrange("p (a b) -> p a b", a=NH))

        def x8_slice(cp, lo, width):
            # [P, 2, width] view of columns lo..lo+width at c-pair cp
            q = lo // NQ
            assert (lo + width - 1) // NQ == q
            return x8q[q][cp][:, :, lo - q * NQ:lo - q * NQ + width]
        ones8 = const.tile([P, 2, P], FP8, tag="ones")
        nc.vector.memset(ones8[:], 1.0)
        # the whole masked block stays resident (2MB fp8 = 16KB/partition),
        # DMA'd once via the otherwise-idle GpSimd issue queue and reused
        # by both heads.  Four j-quarter tiles so the first masked multiply
        # only waits on the quarter it reads (per-tile dependencies).
        MQ = MJ // 4
        mk_sb = [const.tile([P, MQ, mm], FP8, tag=f"mk{i}", name=f"mk{i}")
                 for i in range(4)]
        for j in range(MJ):
            nc.gpsimd.dma_start(mk_sb[j // MQ][:, j % MQ, :], mk_v[:, j, :])

        # Resident intermediates
        qt = [const.tile([P, n], BF16, tag=f"qt{h}", name=f"qt{h}") for h in range(NH)]
        kt = [const.tile([P, n], BF16, tag=f"kt{h}", name=f"kt{h}") for h in range(NH)]
        vb8 = const.tile([P, JC, NH * DH], FP8, tag="vb")      # [j, jc, (h d)]
        ot = [const.tile([P, n], BF16, tag=f"ot{h}", name=f"ot{h}") for h in range(NH)]

        pst = ctx.enter_context(tc.tile_pool(name="pst", bufs=2, space="PSUM"))
        px = ctx.enter_context(tc.tile_pool(name="px", bufs=2, space="PSUM"))
        po = ctx.enter_context(tc.tile_pool(name="po", bufs=1, space="PSUM"))
        pd = ctx.enter_context(tc.tile_pool(name="pd", bufs=1, space="PSUM"))
        att = ctx.enter_context(tc.tile_pool(name="att", bufs=8))
        obp = ctx.enter_context(tc.tile_pool(name="obp", bufs=3))

        # ---- emission units (each: a few PE streams + a DVE eviction) ----
        def emit_qk_g(h, w8, dst, g):
            # one i-group of a Q^T/K^T projection: DR over c-chunk pairs
            ps = px.tile([P, FD], F32, tag="u", name="psu")
            for cp in range(CP):
                nc.tensor.matmul(
                    ps[:], w8[cp][:, :, h * DH:(h + 1) * DH],
                    x8_slice(cp, g * FD, FD),
                    start=(cp == 0), stop=(cp == CP - 1), perf_mode=DR,
                )
            nc.vector.tensor_copy(dst[:, g * FD:(g + 1) * FD], ps[:])

        def emit_v_chunk(t):
            # one 128-row chunk of V for both heads, evicted to fp8.
            # lhsT is the fp8 x (the PE takes mixed fp8 weights x bf16
            # ifmap); wv stays bf16 so V only carries x's quantization.
            ps = px.tile([P, FD], F32, tag="u", name="psu")
            pv = ps[:, :NH * DH]
            q, col = t * P // NQ, t * P % NQ
            for c in range(CI):
                nc.tensor.matmul(
                    pv, x8q[q][c // 2][:, c % 2, col:col + P], wv[:, c, :],
                    start=(c == 0), stop=(c == CI - 1),
                )
            nc.vector.tensor_copy(vb8[:, t, :], pv)

        def emit_outproj_half(t, nf, split=1):
            ps = px.tile([P, FD], F32, tag="u", name="psu")
            for h in range(NH):
                nc.tensor.matmul(
                    ps[:], ot[h][:, t * P:(t + 1) * P],
                    wo[:, h, nf * FD:(nf + 1) * FD],
                    start=(h == 0), stop=(h == NH - 1),
                )
            ob = obp.tile([P, FD], BF16, tag="ob", name="ob")
            nc.vector.tensor_copy(ob[:], ps[:])
            w = FD // split
            for s in range(split):
                nc.sync.dma_start(
                    out_v[t][:, nf * FD + s * w:nf * FD + (s + 1) * w],
                    ob[:, s * w:(s + 1) * w])

        # splice queues, drained on a fixed schedule inside the pair loop
        qk1_units = deque()
        op_units = deque()

        # ---- head: Q/K projections for head 0, first V chunks ----
        # q then k over the FIRST x8 half, then q/k over the second half:
        # later x8 quarters get twice the time to land before their first
        # consumer (each quarter feeds two g's of both q and k)
        for g0 in (0, IG // 2):
            for w8, dst in ((wq8, qt[0]), (wk8, kt[0])):
                for g in range(g0, g0 + IG // 2):
                    emit_qk_g(0, w8, dst, g)
        V_UPFRONT = 4
        for t in range(V_UPFRONT):
            emit_v_chunk(t)
        v_todo = deque(range(V_UPFRONT, JC))
        for w8, dst in ((wq8, qt[1]), (wk8, kt[1])):
            for g in range(IG):
                qk1_units.append(lambda h=1, w8=w8, dst=dst, g=g:
                                 emit_qk_g(h, w8, dst, g))

        # ---- attention pair loop (phases interleaved via splice pops) ----
        NP2 = JC // 2
        h0_pair = [0]

        def pop_splices(h, g, jp):
            if h == 0 and g == 0:
                # V chunks just-in-time, one pair ahead of this PV stream
                for _ in range(2):
                    if v_todo:
                        emit_v_chunk(v_todo.popleft())
            elif h == 0:
                # head-1 Q/K projection spread evenly over these 112 pairs
                h0_pair[0] += 1
                if h0_pair[0] % 7 == 3 and qk1_units:
                    qk1_units.popleft()()
            else:
                # output projection halves, one group's lag behind finalize;
                # drain faster near the end so the tail stays short
                if (jp % 2 == 1 or g >= IG - 2) and op_units:
                    op_units.popleft()()

        # One flat stream of all 256 (h, g, jp) pairs with S-emission two
        # pairs ahead of consumption: the exp pipe on ScalarE runs back to
        # back straight through group boundaries (no per-group bubble).
        pairs = [(h, g, jp)
                 for h in range(NH) for g in range(IG) for jp in range(NP2)]

        def emit_s(h, g, jp):
            st = pst.tile([P, 2, FD], F32, tag="st", name="st")
            for u in range(2):
                nc.tensor.matmul(
                    st[:, u, :],
                    kt[h][:, (2 * jp + u) * P:(2 * jp + u + 1) * P],
                    qt[h][:, g * FD:(g + 1) * FD],
                    start=True, stop=True,
                )
            return st

        st_q = deque(emit_s(*pairs[i]) for i in range(2))
        oacc = dacc = None
        qd = pend_qd = None
        den_started = False
        for idx, (h, g, jp) in enumerate(pairs):
            gs = g * FD
            if jp == 0:
                oacc = po.tile([P, FD], F32, tag="po", name="oacc")
                dacc = pd.tile([P, FD], F32, tag="pd", name="dacc")
                den_started = False
                qd = pend_qd = None
            st = st_q.popleft()
            if idx + 2 < len(pairs):
                st_q.append(emit_s(*pairs[idx + 2]))
            j0 = 2 * jp
            masked = j0 + 1 < MJ and g < MG
            # in fully-unmasked groups the denominator is folded one more
            # level: DVE pre-sums each exp pair (fp8) so one ones-weights
            # DoubleRow matmul covers FOUR j-chunks (halves den PE streams)
            quadfold = g >= MG
            pt2 = att.tile([P, 2, FD], FP8, tag="pt")
            nc.scalar.activation(pt2[:], st[:], AF.Exp, scale=SCALE)
            if masked:
                nc.vector.tensor_mul(
                    out=pt2[:], in0=pt2[:],
                    in1=mk_sb[j0 // MQ][:, j0 % MQ:j0 % MQ + 2, gs:gs + FD])
            last_pair = jp == NP2 - 1
            if quadfold:
                # pair-sum ahead of the splice evictions in the DVE queue,
                # so the consuming den matmul never waits on it
                if jp % 2 == 0:
                    qd = att.tile([P, 2, FD], FP8, tag="qd", bufs=3, name="qd")
                nc.vector.tensor_add(
                    out=qd[:, jp % 2, :], in0=pt2[:, 0, :], in1=pt2[:, 1, :])
            pop_splices(h, g, jp)
            if pend_qd is not None:
                # quad from pairs (jp-2, jp-1): its DVE sums are long done
                nc.tensor.matmul(
                    dacc[:], ones8[:], pend_qd[:],
                    start=not den_started, stop=False, perf_mode=DR,
                )
                den_started = True
                pend_qd = None
            nc.tensor.matmul(
                oacc[:], vb8[:, j0:j0 + 2, h * DH:(h + 1) * DH],
                pt2[:], start=(jp == 0), stop=last_pair,
                perf_mode=DR,
            )
            if quadfold:
                if jp % 2 == 1:
                    pend_qd = qd
                if last_pair:
                    nc.tensor.matmul(
                        dacc[:], ones8[:], pend_qd[:],
                        start=not den_started, stop=True, perf_mode=DR,
                    )
                    pend_qd = None
            else:
                nc.tensor.matmul(
                    dacc[:], ones8[:], pt2[:],
                    start=(jp == 0), stop=last_pair,
                    perf_mode=DR,
                )
            if last_pair:
                # free the single-bank accumulators ASAP, then normalize
                osb = att.tile([P, FD], F32, tag="osb", name="osb", bufs=2)
                dsb = att.tile([P, FD], F32, tag="dsb", name="dsb", bufs=2)
                nc.vector.tensor_copy(osb[:], oacc[:])
                nc.vector.tensor_copy(dsb[:], dacc[:])
                rec = att.tile([P, FD], F32, tag="rec", name="rec", bufs=2)
                nc.vector.reciprocal_approx_fast(rec[:], dsb[:])
                nc.vector.tensor_mul(
                    out=ot[h][:, gs:gs + FD], in0=osb[:], in1=rec[:])
                if h == NH - 1:
                    for t in range(4 * g, 4 * g + 4):
                        for nf in range(OUT_DIM // FD):
                            op_units.append(lambda split=1, t=t, nf=nf:
                                            emit_outproj_half(t, nf, split))

        # ---- tail: drain remaining spliced work ----
        while v_todo:
            emit_v_chunk(v_todo.popleft())
        while qk1_units:
            qk1_units.popleft()()
        while op_units:
            op_units.popleft()()

    nc.compile()
    return nc


def make_core_inputs(x, W_qkv, W_out, mask, n=N_FULL, mm=MM_FULL):
    """Host-side shard prep: per-core input dicts (pre-transposed).

    W slices are delivered in the on-chip layout ([128, c*h*d] with the
    IN_DIM chunk index between partition and column) so the DMA is dense.
    x ships twice: fp8 (Q/K DoubleRow path) and bf16 (V path).
    """
    bf = ml_dtypes.bfloat16
    f8 = ml_dtypes.float8_e4m3
    B = x.shape[0]
    CI = IN_DIM // P
    xt_f8 = [np.ascontiguousarray(x[b].T).astype(f8) for b in range(B)]
    maskt = np.ascontiguousarray(mask[0, 0, :mm, :mm].T).astype(f8)

    def wlayout(w, dt):  # [IN_DIM, NH*DH] -> [P, CI*NH*DH]
        return np.ascontiguousarray(
            w.reshape(CI, P, NH * DH).transpose(1, 0, 2).reshape(P, -1)
        ).astype(dt)

    cores_per_b = N_CORES // B
    in_maps = []
    for core in range(N_CORES):
        b = core // cores_per_b
        h0 = NH * (core % cores_per_b)
        qs, ks, vs = (W_qkv[:, o + h0 * DH: o + (h0 + NH) * DH]
                      for o in (0, OUT_DIM, 2 * OUT_DIM))
        wo_slice = W_out[h0 * DH:(h0 + NH) * DH, :]  # [NH*DH, OUT_DIM]
        wo_l = np.ascontiguousarray(
            wo_slice.reshape(NH, P, OUT_DIM).transpose(1, 0, 2).reshape(P, -1)
        ).astype(bf)
        in_maps.append({
            "x8": xt_f8[b],
            "wq": wlayout(qs, f8),
            "wk": wlayout(ks, f8),
            "wv": wlayout(vs, bf),
            "wo": wo_l,
            "maskt": maskt,
        })
    return in_maps


_NC_CACHE = {}


def _get_nc(n=N_FULL, mm=MM_FULL):
    key = (n, mm)
    if key not in _NC_CACHE:
        _NC_CACHE[key] = build_nc(n, mm)
    return _NC_CACHE[key]


def run(x, W_qkv, W_out, b_out, mask, trace=False, **trace_kwargs):
    nc = _get_nc()
    in_maps = make_core_inputs(x, W_qkv, W_out, mask)
    res = run_bass_kernel_spmd(
        nc, in_maps, list(range(N_CORES)), trace=trace, **trace_kwargs
    )
    B = x.shape[0]
    cores_per_b = N_CORES // B
    out = np.zeros((B, N_FULL, OUT_DIM), np.float32)
    for core in range(N_CORES):
        out[core // cores_per_b] += np.asarray(
            res.results[core]["part"], dtype=np.float32)
    out += np.asarray(b_out, np.float32)
    return out, res


def kernel(x, W_qkv, W_out, b_out, mask, max_mask=MM_FULL, **_ignored):
    x = np.asarray(x, np.float32)
    W_qkv = np.asarray(W_qkv, np.float32)
    W_out = np.asarray(W_out, np.float32)
    b_out = np.asarray(b_out, np.float32)
    mask = np.asarray(mask)
    out, _ = run(x, W_qkv, W_out, b_out, mask)
    return out
